# revision 22
# baseline (speedup 1.0000x reference)
"""Trainium2 Bass kernel for MiniPointNet (segment_reduce) — v3.

Data-parallel over batch (32 -> 8 cores x 4). Host-side counting sort with a
runtime-computed tiered slot table: clusters are rank-ordered by size per
batch element and rank r gets a fixed slot count S_r (multiple of 16, shared
across the batch), so the padded point count NP2 (~9216) is much smaller than
the uniform-176 padding (11264). The device kernel is fully static dense
compute, software-pipelined at 512-point chunks:

  mm1+BN+relu -> mm2 -> 16-block max off PSUM -> tiered cluster max -> fg
  -> zt = fg @ W3g'(s2-scaled) + t2 (via scalar_tensor_tensor), duplicated
     across both partition halves
  -> mm3 (s2 folded into W3c; grouped MMs) + one-hot zt add as two row-packed
     MM pairs (rows 0-63 / 64-127 run concurrently) -> single wide relu ACT
     over 2-bank PSUM tiles -> mm4 -> block max -> tiered cluster max -> +b4
  -> out planes; host un-permutes cluster ranks.

No gpsimd, no data-dependent addressing, no element-granular DMA.
"""
import numpy as np

import concourse.bass as bass
import concourse.tile as tile
from concourse import bacc, mybir
from contextlib import ExitStack

F32 = mybir.dt.float32
BF16 = mybir.dt.bfloat16

B, N, K, OUT = 32, 8192, 64, 384
EPS = 1e-5
N_CORES = 8
B_LOC = B // N_CORES
CHW = 512


def _tier_table(counts):
    """counts [B,K] -> (tiers tuple of (n_ranks, slot_pts), NP2 rounded)."""
    import itertools
    env = np.sort(counts, axis=1)[:, ::-1].max(axis=0)
    S_at = (np.ceil(env / 16.0).astype(np.int64) * 16).astype(int)
    Kk = len(env)
    best = None
    for T in (1, 2, 3, 4):
        for bounds in itertools.combinations(range(1, Kk), T - 1):
            bs = (0,) + bounds + (Kk,)
            tiers, tot = [], 0
            for a, b2 in zip(bs, bs[1:]):
                S = int(S_at[a])
                tiers.append((b2 - a, S))
                tot += (b2 - a) * S
            np2 = -(-tot // (2 * CHW)) * (2 * CHW)
            key = (np2, T)
            if best is None or key < best[0]:
                best = (key, tuple(tiers), np2)
    return best[1], best[2]


def build(nc, tiers, np2, b_loc=B_LOC):
    NCH = np2 // CHW
    NB = np2 // 16
    tspec = []          # (rank0, n_ranks, blocks_per_rank, block0)
    r0 = blk0 = 0
    for n, S in tiers:
        g = S // 16
        tspec.append((r0, n, g, blk0))
        r0 += n
        blk0 += n * g

    dt = lambda name, shape, kind="ExternalInput", d=F32: nc.dram_tensor(
        name, shape, d, kind=kind).ap()

    xyzS = dt("xyzS", [b_loc, 8, np2 // 2], d=BF16)
    w1x = dt("w1x", [36, 128], d=BF16)
    w2t = dt("w2t", [128, 256], d=BF16)
    w3ct = dt("w3ct", [128, 512], d=BF16)
    w3gt = dt("w3gt", [128, 2, 512], d=BF16)
    w4t = dt("w4t", [128, 4, 384], d=BF16)
    s1t1 = dt("s1t1", [128, 2])
    t2r = dt("t2r", [128, 512])
    c4c = dt("c4c", [128, 3])
    ohf = dt("ohf", [128, np2], d=BF16)
    out_d = dt("out", [b_loc, 3, 128, 64], kind="ExternalOutput")

    AX = mybir.AxisListType.X
    OP = mybir.AluOpType
    RELU = mybir.ActivationFunctionType.Relu

    with tile.TileContext(nc) as tc, ExitStack() as ctx:
        cpool = ctx.enter_context(tc.tile_pool(name="consts", bufs=1))
        big = ctx.enter_context(tc.tile_pool(name="big", bufs=1))
        h2p = ctx.enter_context(tc.tile_pool(name="h2p", bufs=3))
        sm = ctx.enter_context(tc.tile_pool(name="sm", bufs=2))

        # two point-halves live at partition bases 0 and 32 so mm1 for chunk
        # pairs (c, c+NCH/2) runs as two concurrent PE row-tiles
        xyzb = [big.tile([36, np2 // 2], BF16, tag=f"xyzb{p}", name=f"xyzb{p}")
                for p in range(2)]

        def load_xyz(b):
            nc.sync.dma_start(xyzb[b % 2][0:4, :], xyzS[b, 0:4])
            nc.sync.dma_start(xyzb[b % 2][32:36, :], xyzS[b, 4:8])

        def load_const(ap_, shape, d=F32):
            t = cpool.tile(shape, d, tag=ap_.tensor.name, name=ap_.tensor.name + "_s")
            nc.sync.dma_start(t[:], ap_[:])
            return t

        # DMA queue order = dependency order of the head: mm1 needs
        # xyz + w1x + s1t1 first; ohf (2.3MB) goes last.
        load_xyz(0)
        w1x_s = load_const(w1x, [36, 128], d=BF16)
        s1t1_s = load_const(s1t1, [128, 2])
        w2t_s = load_const(w2t, [128, 256], d=BF16)
        if b_loc > 1:
            load_xyz(1)
        w3ct_s = load_const(w3ct, [128, 512], d=BF16)
        w3gt_s = load_const(w3gt, [128, 2, 512], d=BF16)
        w4t_s = load_const(w4t, [128, 4, 384], d=BF16)
        t2r_s = load_const(t2r, [128, 512])
        c4c_s = load_const(c4c, [128, 3])
        ohf_s = load_const(ohf, [128, np2], d=BF16)

        bm1 = [[big.tile([128, NB], F32, tag=f"bm1_{p}_{o2}", name=f"bm1_{p}_{o2}")
                for o2 in range(2)] for p in range(2)]
        bm2 = [[big.tile([128, NB], F32, tag=f"bm2_{p}_{j}", name=f"bm2_{p}_{j}")
                for j in range(3)] for p in range(2)]
        h1f = [big.tile([128, np2], BF16, tag=f"h1f{p}", name=f"h1f{p}")
               for p in range(2)]

        def p2_s1(b, cp, pool, tag="mp"):
            """mm1 for chunk pair (cp, cp + NCH/2) as two concurrent row-tiles."""
            sl = slice(cp * CHW, (cp + 1) * CHW)
            pss = []
            for half, pb in ((0, 0), (1, 32)):
                ps = pool.tile([128, CHW], F32, tag=tag, name="h1ps")
                nc.tensor.matmul(out=ps[:], lhsT=w1x_s[pb:pb + 4, :],
                                 rhs=xyzb[b % 2][pb:pb + 4, sl],
                                 start=True, stop=True)
                pss.append(ps)
            for half, ps in enumerate(pss):
                c = cp + half * (NCH // 2)
                nc.scalar.activation(out=h1f[b % 2][:, c * CHW:(c + 1) * CHW],
                                     in_=ps[:], func=RELU,
                                     bias=s1t1_s[:, 1:2], scale=s1t1_s[:, 0:1])

        def p2_s2(b, c, pool, tag="mp"):
            sl = slice(c * CHW, (c + 1) * CHW)
            for o2 in range(2):
                ps = pool.tile([128, CHW], F32, tag=tag, name="f1ps")
                nc.tensor.matmul(out=ps[:],
                                 lhsT=w2t_s[:, o2 * 128:(o2 + 1) * 128],
                                 rhs=h1f[b % 2][:, sl], start=True, stop=True)
                nc.vector.tensor_reduce(
                    out=bm1[b % 2][o2][:, c * 32:(c + 1) * 32],
                    in_=ps[:].rearrange("p (a s) -> p a s", s=16),
                    axis=AX, op=OP.max)



        def fg_reduce(b, o2):
            fgt = sm.tile([128, 64], BF16, tag=f"fg{o2}", name=f"fg{o2}_{b}")
            for (tr0, n, g, tb0) in tspec:
                nc.vector.tensor_reduce(
                    out=fgt[:, tr0:tr0 + n],
                    in_=bm1[b % 2][o2][:, tb0:tb0 + n * g].rearrange(
                        "p (k g) -> p k g", g=g),
                    axis=AX, op=OP.max)
            return fgt

        def zt_compute(b, fg):
            ztps = psm.tile([128, 512], F32, tag="mp", name="ztps")
            for cg in (0, 64):
                for o2 in range(2):
                    nc.tensor.matmul(out=ztps[cg:cg + 64, :], lhsT=fg[o2][:],
                                     rhs=w3gt_s[:, o2, :],
                                     start=(o2 == 0), stop=(o2 == 1))
            ztd = sm.tile([128, 512], BF16, tag="ztd", name=f"ztd_{b}")
            nc.vector.scalar_tensor_tensor(out=ztd[:], in0=ztps[:], scalar=1.0,
                                           in1=t2r_s[:], op0=OP.mult, op1=OP.add)
            return ztd

        def p5_s1(b, c, ztd, mid=None):
            sl = slice(c * CHW, (c + 1) * CHW)
            pa = ps1.tile([128, 1024], F32, tag="s1a", name="s1a")
            pb = ps1.tile([128, 1024], F32, tag="s1b", name="s1b")
            halves = [(pa, 0), (pa, 1), (pb, 0), (pb, 1)]
            for i in range(4):
                t, h = halves[i]
                nc.tensor.matmul(out=t[:, h * 512:(h + 1) * 512],
                                 lhsT=w3ct_s[:, i * 128:(i + 1) * 128],
                                 rhs=h1f[b % 2][:, sl], start=True, stop=False)
            for i in range(4):
                t, h = halves[i]
                rg = (i % 2) * 64
                nc.tensor.matmul(out=t[:, h * 512:(h + 1) * 512],
                                 lhsT=ztd[rg:rg + 64, i * 128:(i + 1) * 128],
                                 rhs=ohf_s[rg:rg + 64, sl],
                                 start=False, stop=True)
            if mid is not None:
                # interleaved mm1-pair: its q-row LDW overlaps the in-flight
                # h-row oh MMs, and its ACTs precede the wide h2 ACTs so its
                # PSUM banks free early for the mm4/mm2 rotation.
                mid()
            h2c = []
            for k2, t in ((0, pa), (1, pb)):
                hc = h2p.tile([128, 1024], BF16, tag=f"h2c{k2}", name=f"h2c{k2}")
                nc.scalar.activation(out=hc[:], in_=t[:], func=RELU)
                h2c.append(hc)
            return h2c

        def p5_s2(b, c, h2c):
            for j in range(3):
                ps = psm.tile([128, CHW], F32, tag="mp", name="f2ps")
                for i in range(4):
                    nc.tensor.matmul(out=ps[:],
                                     lhsT=w4t_s[:, i, j * 128:(j + 1) * 128],
                                     rhs=h2c[i // 2][:, (i % 2) * 512:(i % 2 + 1) * 512],
                                     start=(i == 0), stop=(i == 3))
                nc.vector.tensor_reduce(
                    out=bm2[b % 2][j][:, c * 32:(c + 1) * 32],
                    in_=ps[:].rearrange("p (a s) -> p a s", s=16),
                    axis=AX, op=OP.max)

        def gen_seg_ops(b):
            ops = []
            for j in range(3):
                def red(b=b, j=j):
                    fg2 = sm.tile([128, 64], F32, tag="fg2", name=f"fg2_{b}_{j}")
                    for (tr0, n, g, tb0) in tspec:
                        nc.vector.tensor_reduce(
                            out=fg2[:, tr0:tr0 + n],
                            in_=bm2[b % 2][j][:, tb0:tb0 + n * g].rearrange(
                                "p (k g) -> p k g", g=g),
                            axis=AX, op=OP.max)
                    return fg2

                def emit(b=b, j=j, red=red):
                    fg2 = red()
                    fg2b = sm.tile([128, 64], F32, tag="fg2b", name=f"fg2b_{b}_{j}")
                    nc.vector.tensor_scalar(out=fg2b[:], in0=fg2[:],
                                            scalar1=c4c_s[:, j:j + 1],
                                            scalar2=None, op0=OP.add)
                    nc.sync.dma_start(out_d[b, j], fg2b[:])
                ops.append(emit)
            return ops

        # ─── HEAD: p2(b0), software-pipelined. mm1's PSUM (drained by the
        # 776ns h1 ACT) gets its own 4-bank rotation so mm2/reduce never
        # serialize behind it; s2 lags s1 by 2 chunks. ───
        with tc.tile_pool(name="psh", bufs=4, space="PSUM") as psh:
            H = NCH // 2
            ready = []
            for k in range(H):
                p2_s1(0, k, psh, tag="h1")
                ready += [k, k + H]
                if k >= 2:
                    p2_s2(0, ready.pop(0), psh, tag="f1")
                    p2_s2(0, ready.pop(0), psh, tag="f1")
            for c in ready:
                p2_s2(0, c, psh, tag="f1")

        ps1 = ctx.enter_context(tc.tile_pool(name="ps1", bufs=1, space="PSUM"))
        psm = ctx.enter_context(tc.tile_pool(name="psm", bufs=4, space="PSUM"))

        fg0 = [fg_reduce(0, o2) for o2 in range(2)]
        ztd_cur = zt_compute(0, fg0)

        # Flat slot stream: slot (b,c) = [p5_s1(b,c); p5_s2(prev slot)] where
        # prev of (b+1,0) is (b,NCH-1) — mm4 of b's last chunk covers the ACT
        # latency of b+1's first chunk across the batch boundary.
        seg_ops = []
        pend = None
        ztd_next = None
        H = NCH // 2
        for b in range(b_loc):
            has_p2 = b + 1 < b_loc
            # p2(b+1): mm1 pair k runs in slot k's mid hook; its two mm2
            # chunks (k, k+H) become poppable at slot k+2 (so mm2 never waits
            # on the h1 relu ACT, which queues behind the wide h2 ACTs).
            s2_ready = []
            n_s2 = 0
            target = NCH - 4
            fgn = [None, None]
            for c in range(NCH):
                mid = None
                if has_p2 and c < H:
                    mid = (lambda b1=b + 1, k=c: p2_s1(b1, k, psm))
                h2 = p5_s1(b, c, ztd_cur, mid=mid)
                if has_p2 and c < H:
                    s2_ready += [c, c + H]
                if has_p2 and c == NCH - 1:
                    # zt MMs + the ztd add land before the last p5_s2's
                    # reduces, so ztd is ready well before (b+1, 0)'s oh MMs.
                    ztd_next = zt_compute(b + 1, fgn)
                if pend is not None:
                    p5_s2(*pend)
                    # segout of b-1 pops on late (DVE-cool) slots only
                    if seg_ops and (not has_p2 or c >= NCH - 7):
                        seg_ops.pop(0)()
                if has_p2 and c >= 2:
                    avail = [x for x in s2_ready if (x % H) + 2 <= c]
                    rem = NCH - n_s2
                    want = -(-rem // max(1, target - c + 1)) if c <= target else rem
                    for x in avail[:want]:
                        s2_ready.remove(x)
                        p2_s2(b + 1, x, psm)
                        n_s2 += 1
                if has_p2:
                    if c == NCH - 3:
                        assert n_s2 == NCH, (n_s2, NCH)
                        fgn[0] = fg_reduce(b + 1, 0)
                    elif c == NCH - 2:
                        fgn[1] = fg_reduce(b + 1, 1)
                pend = (b, c, h2)
            seg_ops = gen_seg_ops(b)
            if b + 2 < b_loc:
                load_xyz(b + 2)
            ztd_cur = ztd_next
        p5_s2(*pend)
        for op in seg_ops:
            op()

    return nc


def fold_weights(W1, b1, g1, be1, m1, v1, W2, b2, W3, b3, g2, be2, m2, v2, W4, b4):
    """Host-side constant folding. BN2's scale s2 is folded into W3c/W3g and
    its bias t2 rides along with zt (added to every zt row on device)."""
    f = np.float32
    s1 = (g1 / np.sqrt(v1 + EPS)).astype(f)
    t1 = (s1 * (b1 - m1) + be1).astype(f)
    W3g = W3[:, :256].astype(np.float64)
    W3f = W3[:, 256:].astype(np.float64)
    c3 = (b3.astype(np.float64) + (W3f + W3g) @ b2.astype(np.float64)).astype(f)
    s2 = (g2 / np.sqrt(v2 + EPS)).astype(f)
    t2 = (s2 * (c3 - m2) + be2).astype(f)
    W3c = (s2[:, None].astype(np.float64) * (W3f @ W2.astype(np.float64))).astype(f)
    W3gs = (s2[:, None] * W3g.astype(f)).astype(f)
    c4 = b4.astype(f)

    import ml_dtypes
    bf = ml_dtypes.bfloat16
    w1x = np.zeros((36, 128), f)
    w1x[0:3, :] = W1.T
    w1x[32:35, :] = W1.T
    return {
        "w1x": w1x.astype(bf),
        "w2t": np.ascontiguousarray(W2.T.astype(f)).astype(bf),
        "w3ct": np.ascontiguousarray(W3c.T).astype(bf),
        "w3gt": np.ascontiguousarray(
            W3gs.T.reshape(2, 128, 512).transpose(1, 0, 2)).astype(bf),
        "w4t": np.ascontiguousarray(
            W4.T.astype(f).reshape(4, 128, 384).transpose(1, 0, 2)).astype(bf),
        "s1t1": np.stack([s1, t1], axis=1),
        "t2r": np.broadcast_to(t2, (128, 512)).copy(),
        "c4c": np.ascontiguousarray(c4.reshape(3, 128).T),
    }


def build_ohf(Sr, starts, np2):
    import ml_dtypes
    oh = np.zeros((128, np2), np.float32)
    for r in range(K):
        oh[r, starts[r]:starts[r] + Sr[r]] = 1.0
    oh[64:128] = oh[0:64]
    return oh.astype(ml_dtypes.bfloat16)


def make_core_inputs(xyz, choice, wf, ohf, Sr, starts, np2):
    """Per-core inputs: rank-slot-sorted transposed xyz + rank orders."""
    import ml_dtypes
    b_loc = xyz.shape[0]
    xs = np.zeros((b_loc, 8, np2 // 2), ml_dtypes.bfloat16)
    orders = []
    np2_raw = int(starts[-1] + Sr[-1])
    idx = np.empty(np2_raw, np.int64)
    xt = np.zeros((4, np2), np.float32)
    for bb in range(b_loc):
        ch = choice[bb]
        cnts = np.bincount(ch, minlength=K)
        order = np.argsort(-cnts, kind="stable")
        ord_pts = np.argsort(ch, kind="stable")
        cum = np.zeros(K + 1, np.int64)
        cum[1:] = np.cumsum(cnts)
        for r in range(K):
            kcl = order[r]
            c0, c1 = cum[kcl], cum[kcl + 1]
            n = c1 - c0
            assert 1 <= n <= Sr[r], (r, n, Sr[r])
            s0 = starts[r]
            idx[s0:s0 + n] = ord_pts[c0:c1]
            idx[s0 + n:s0 + Sr[r]] = ord_pts[c1 - 1]
        xt[:3, :np2_raw] = xyz[bb][idx].T
        xs[bb, 0:4, :] = xt[:, :np2 // 2].astype(ml_dtypes.bfloat16)
        xs[bb, 4:8, :] = xt[:, np2 // 2:].astype(ml_dtypes.bfloat16)
        orders.append(order)
    m = {"xyzS": xs, "ohf": ohf}
    m.update(wf)
    return m, orders


_BUILT = {}


def get_built(tiers, np2, b_loc=B_LOC):
    key = (tiers, np2, b_loc)
    if key not in _BUILT:
        nc = bacc.Bacc("TRN2", target_bir_lowering=False, debug=False,
                       num_devices=N_CORES if b_loc == B_LOC else 1)
        build(nc, tiers, np2, b_loc)
        nc.compile()
        _BUILT[key] = nc
    return _BUILT[key]


def prepare(inputs):
    """Returns (nc, in_maps, post) where post(results) -> full output."""
    xyz = np.asarray(inputs["normalized_xyz"], np.float32)
    choice = np.asarray(inputs["choice"], np.int32)
    wf = fold_weights(**{k: np.asarray(inputs[k], np.float32) for k in
                         ["W1", "b1", "g1", "be1", "m1", "v1", "W2", "b2",
                          "W3", "b3", "g2", "be2", "m2", "v2", "W4", "b4"]})
    cnts = np.stack([np.bincount(choice[b], minlength=K) for b in range(B)])
    assert cnts.min() >= 1
    tiers, np2 = _tier_table(cnts)
    Sr = np.concatenate([[S] * n for n, S in tiers]).astype(np.int64)
    starts = np.zeros(K, np.int64)
    starts[1:] = np.cumsum(Sr)[:-1]
    nc = get_built(tiers, np2)
    ohf = build_ohf(Sr, starts, np2)
    in_maps, all_orders = [], []
    for core in range(N_CORES):
        sl = slice(core * B_LOC, (core + 1) * B_LOC)
        m, orders = make_core_inputs(xyz[sl], choice[sl], wf, ohf, Sr, starts, np2)
        in_maps.append(m)
        all_orders.append(orders)

    def post(results):
        out = np.empty((B, K, OUT), np.float32)
        for core in range(N_CORES):
            arr = np.asarray(results[core]["out"], np.float32)
            arr = arr.transpose(0, 3, 1, 2).reshape(B_LOC, K, OUT)
            for bb in range(B_LOC):
                out[core * B_LOC + bb, all_orders[core][bb]] = arr[bb]
        return out

    return nc, in_maps, post


def kernel(**inputs):
    from concourse.bass_utils import run_bass_kernel_spmd
    nc, in_maps, post = prepare(inputs)
    res = run_bass_kernel_spmd(nc, in_maps, core_ids=list(range(N_CORES)))
    return post(res.results)


# revision 24
# speedup vs baseline: 1.1880x; 1.1880x over previous
"""Trainium2 Bass kernel for MiniPointNet (segment_reduce) — v3.

Data-parallel over batch (32 -> 8 cores x 4). Host-side counting sort with a
runtime-computed tiered slot table: clusters are rank-ordered by size per
batch element and rank r gets a fixed slot count S_r (multiple of 16, shared
across the batch), so the padded point count NP2 (~9216) is much smaller than
the uniform-176 padding (11264). The device kernel is fully static dense
compute, software-pipelined at 512-point chunks:

  mm1+BN+relu -> mm2 -> 16-block max off PSUM -> tiered cluster max -> fg
  -> zt = fg @ W3g'(s2-scaled) + t2 (via scalar_tensor_tensor), duplicated
     across both partition halves
  -> mm3 (s2 folded into W3c; grouped MMs) + one-hot zt add as two row-packed
     MM pairs (rows 0-63 / 64-127 run concurrently) -> single wide relu ACT
     over 2-bank PSUM tiles -> mm4 -> block max -> tiered cluster max -> +b4
  -> out planes; host un-permutes cluster ranks.

No gpsimd, no data-dependent addressing, no element-granular DMA.
"""
import numpy as np

import concourse.bass as bass
import concourse.tile as tile
from concourse import bacc, mybir
from contextlib import ExitStack

F32 = mybir.dt.float32
BF16 = mybir.dt.bfloat16

B, N, K, OUT = 32, 8192, 64, 384
EPS = 1e-5
N_CORES = 8
B_LOC = B // N_CORES
CHW = 512


def _tier_table(counts):
    """counts [B,K] -> (tiers tuple of (n_ranks, slot_pts), NP2 rounded)."""
    import itertools
    env = np.sort(counts, axis=1)[:, ::-1].max(axis=0)
    S_at = (np.ceil(env / 16.0).astype(np.int64) * 16).astype(int)
    Kk = len(env)
    best = None
    for T in (1, 2, 3, 4):
        for bounds in itertools.combinations(range(1, Kk), T - 1):
            bs = (0,) + bounds + (Kk,)
            tiers, tot = [], 0
            for a, b2 in zip(bs, bs[1:]):
                S = int(S_at[a])
                tiers.append((b2 - a, S))
                tot += (b2 - a) * S
            np2 = -(-tot // (2 * CHW)) * (2 * CHW)
            key = (np2, T)
            if best is None or key < best[0]:
                best = (key, tuple(tiers), np2)
    return best[1], best[2]


def build(nc, tiers, np2, b_loc=B_LOC):
    NCH = np2 // CHW
    NB = np2 // 16
    tspec = []          # (rank0, n_ranks, blocks_per_rank, block0)
    r0 = blk0 = 0
    for n, S in tiers:
        g = S // 16
        tspec.append((r0, n, g, blk0))
        r0 += n
        blk0 += n * g

    dt = lambda name, shape, kind="ExternalInput", d=F32: nc.dram_tensor(
        name, shape, d, kind=kind).ap()

    xyzS = dt("xyzS", [b_loc, 8, np2 // 2], d=BF16)
    w1x = dt("w1x", [36, 128], d=BF16)
    w2t = dt("w2t", [128, 256], d=BF16)
    w3ct = dt("w3ct", [128, 512], d=BF16)
    w3gt = dt("w3gt", [128, 2, 512], d=BF16)
    w4t = dt("w4t", [128, 4, 384], d=BF16)
    s1t1 = dt("s1t1", [128, 2])
    t2r = dt("t2r", [128, 512])
    c4c = dt("c4c", [128, 3])
    ohf = dt("ohf", [128, np2], d=BF16)
    out_d = dt("out", [b_loc, 3, 128, 64], kind="ExternalOutput")

    AX = mybir.AxisListType.X
    OP = mybir.AluOpType
    RELU = mybir.ActivationFunctionType.Relu

    with tile.TileContext(nc) as tc, ExitStack() as ctx:
        cpool = ctx.enter_context(tc.tile_pool(name="consts", bufs=1))
        big = ctx.enter_context(tc.tile_pool(name="big", bufs=1))
        h2p = ctx.enter_context(tc.tile_pool(name="h2p", bufs=3))
        sm = ctx.enter_context(tc.tile_pool(name="sm", bufs=2))

        # two point-halves live at partition bases 0 and 32 so mm1 for chunk
        # pairs (c, c+NCH/2) runs as two concurrent PE row-tiles
        xyzb = [big.tile([36, np2 // 2], BF16, tag=f"xyzb{p}", name=f"xyzb{p}")
                for p in range(2)]

        def load_xyz(b):
            nc.sync.dma_start(xyzb[b % 2][0:4, :], xyzS[b, 0:4])
            nc.sync.dma_start(xyzb[b % 2][32:36, :], xyzS[b, 4:8])

        def load_const(ap_, shape, d=F32):
            t = cpool.tile(shape, d, tag=ap_.tensor.name, name=ap_.tensor.name + "_s")
            nc.sync.dma_start(t[:], ap_[:])
            return t

        # DMA queue order = dependency order of the head: mm1 needs
        # xyz + w1x + s1t1 first; ohf (2.3MB) goes last.
        load_xyz(0)
        w1x_s = load_const(w1x, [36, 128], d=BF16)
        s1t1_s = load_const(s1t1, [128, 2])
        w2t_s = load_const(w2t, [128, 256], d=BF16)
        if b_loc > 1:
            load_xyz(1)
        w3ct_s = load_const(w3ct, [128, 512], d=BF16)
        w3gt_s = load_const(w3gt, [128, 2, 512], d=BF16)
        w4t_s = load_const(w4t, [128, 4, 384], d=BF16)
        t2r_s = load_const(t2r, [128, 512])
        c4c_s = load_const(c4c, [128, 3])
        ohf_s = load_const(ohf, [128, np2], d=BF16)

        bm1 = [[big.tile([128, NB], F32, tag=f"bm1_{p}_{o2}", name=f"bm1_{p}_{o2}")
                for o2 in range(2)] for p in range(2)]
        bm2 = [[big.tile([128, NB], F32, tag=f"bm2_{p}_{j}", name=f"bm2_{p}_{j}")
                for j in range(3)] for p in range(2)]
        h1f = [big.tile([128, np2], BF16, tag=f"h1f{p}", name=f"h1f{p}")
               for p in range(2)]

        def p2_s1(b, cp, pool, tag="mp"):
            """mm1 for chunk pair (cp, cp + NCH/2) as two concurrent row-tiles."""
            sl = slice(cp * CHW, (cp + 1) * CHW)
            pss = []
            for half, pb in ((0, 0), (1, 32)):
                ps = pool.tile([128, CHW], F32, tag=tag, name="h1ps")
                nc.tensor.matmul(out=ps[:], lhsT=w1x_s[pb:pb + 4, :],
                                 rhs=xyzb[b % 2][pb:pb + 4, sl],
                                 start=True, stop=True)
                pss.append(ps)
            for half, ps in enumerate(pss):
                c = cp + half * (NCH // 2)
                nc.scalar.activation(out=h1f[b % 2][:, c * CHW:(c + 1) * CHW],
                                     in_=ps[:], func=RELU,
                                     bias=s1t1_s[:, 1:2], scale=s1t1_s[:, 0:1])

        def p2_s2(b, c, pool, tag="mp"):
            sl = slice(c * CHW, (c + 1) * CHW)
            for o2 in range(2):
                ps = pool.tile([128, CHW], F32, tag=tag, name="f1ps")
                nc.tensor.matmul(out=ps[:],
                                 lhsT=w2t_s[:, o2 * 128:(o2 + 1) * 128],
                                 rhs=h1f[b % 2][:, sl], start=True, stop=True)
                nc.vector.tensor_reduce(
                    out=bm1[b % 2][o2][:, c * 32:(c + 1) * 32],
                    in_=ps[:].rearrange("p (a s) -> p a s", s=16),
                    axis=AX, op=OP.max)



        def fg_reduce(b, o2):
            fgt = sm.tile([128, 64], BF16, tag=f"fg{o2}", name=f"fg{o2}_{b}")
            for (tr0, n, g, tb0) in tspec:
                nc.vector.tensor_reduce(
                    out=fgt[:, tr0:tr0 + n],
                    in_=bm1[b % 2][o2][:, tb0:tb0 + n * g].rearrange(
                        "p (k g) -> p k g", g=g),
                    axis=AX, op=OP.max)
            return fgt

        def zt_compute(b, fg):
            ztps = psm.tile([128, 512], F32, tag="mp", name="ztps")
            for cg in (0, 64):
                for o2 in range(2):
                    nc.tensor.matmul(out=ztps[cg:cg + 64, :], lhsT=fg[o2][:],
                                     rhs=w3gt_s[:, o2, :],
                                     start=(o2 == 0), stop=(o2 == 1))
            ztd = sm.tile([128, 512], BF16, tag="ztd", name=f"ztd_{b}")
            nc.vector.scalar_tensor_tensor(out=ztd[:], in0=ztps[:], scalar=1.0,
                                           in1=t2r_s[:], op0=OP.mult, op1=OP.add)
            return ztd

        def p5_s1(b, c, ztd, mid=None):
            sl = slice(c * CHW, (c + 1) * CHW)
            pa = ps1.tile([128, 1024], F32, tag="s1a", name="s1a")
            pb = ps1.tile([128, 1024], F32, tag="s1b", name="s1b")
            halves = [(pa, 0), (pa, 1), (pb, 0), (pb, 1)]
            for i in range(4):
                t, h = halves[i]
                nc.tensor.matmul(out=t[:, h * 512:(h + 1) * 512],
                                 lhsT=w3ct_s[:, i * 128:(i + 1) * 128],
                                 rhs=h1f[b % 2][:, sl], start=True, stop=False)
            for i in range(4):
                t, h = halves[i]
                rg = (i % 2) * 64
                nc.tensor.matmul(out=t[:, h * 512:(h + 1) * 512],
                                 lhsT=ztd[rg:rg + 64, i * 128:(i + 1) * 128],
                                 rhs=ohf_s[rg:rg + 64, sl],
                                 start=False, stop=True)
            if mid is not None:
                # interleaved mm1-pair: its q-row LDW overlaps the in-flight
                # h-row oh MMs, and its ACTs precede the wide h2 ACTs so its
                # PSUM banks free early for the mm4/mm2 rotation.
                mid()
            h2c = []
            for k2, t in ((0, pa), (1, pb)):
                hc = h2p.tile([128, 1024], BF16, tag=f"h2c{k2}", name=f"h2c{k2}")
                nc.scalar.activation(out=hc[:], in_=t[:], func=RELU)
                h2c.append(hc)
            return h2c

        def p5_s2(b, c, h2c, seg_after=None):
            for j in range(3):
                ps = psm.tile([128, CHW], F32, tag="mp", name="f2ps")
                for i in range(4):
                    nc.tensor.matmul(out=ps[:],
                                     lhsT=w4t_s[:, i, j * 128:(j + 1) * 128],
                                     rhs=h2c[i // 2][:, (i % 2) * 512:(i % 2 + 1) * 512],
                                     start=(i == 0), stop=(i == 3))
                nc.vector.tensor_reduce(
                    out=bm2[b % 2][j][:, c * 32:(c + 1) * 32],
                    in_=ps[:].rearrange("p (a s) -> p a s", s=16),
                    axis=AX, op=OP.max)
                if seg_after is not None:
                    seg_after[j]()

        def gen_seg_ops(b):
            ops = []
            for j in range(3):
                def red(b=b, j=j):
                    fg2 = sm.tile([128, 64], F32, tag="fg2", name=f"fg2_{b}_{j}")
                    for (tr0, n, g, tb0) in tspec:
                        nc.vector.tensor_reduce(
                            out=fg2[:, tr0:tr0 + n],
                            in_=bm2[b % 2][j][:, tb0:tb0 + n * g].rearrange(
                                "p (k g) -> p k g", g=g),
                            axis=AX, op=OP.max)
                    return fg2

                def emit(b=b, j=j, red=red):
                    fg2 = red()
                    fg2b = sm.tile([128, 64], F32, tag="fg2b", name=f"fg2b_{b}_{j}")
                    nc.vector.tensor_scalar(out=fg2b[:], in0=fg2[:],
                                            scalar1=c4c_s[:, j:j + 1],
                                            scalar2=None, op0=OP.add)
                    nc.sync.dma_start(out_d[b, j], fg2b[:])
                ops.append(emit)
            return ops

        # ─── HEAD: p2(b0), software-pipelined. mm1's PSUM (drained by the
        # 776ns h1 ACT) gets its own 4-bank rotation so mm2/reduce never
        # serialize behind it; s2 lags s1 by 2 chunks. ───
        with tc.tile_pool(name="psh", bufs=4, space="PSUM") as psh:
            H = NCH // 2
            ready = []
            for k in range(H):
                p2_s1(0, k, psh, tag="h1")
                ready += [k, k + H]
                if k >= 2:
                    p2_s2(0, ready.pop(0), psh, tag="f1")
                    p2_s2(0, ready.pop(0), psh, tag="f1")
            for c in ready:
                p2_s2(0, c, psh, tag="f1")

        ps1 = ctx.enter_context(tc.tile_pool(name="ps1", bufs=1, space="PSUM"))
        psm = ctx.enter_context(tc.tile_pool(name="psm", bufs=4, space="PSUM"))

        fg0 = [fg_reduce(0, o2) for o2 in range(2)]
        ztd_cur = zt_compute(0, fg0)

        # Flat slot stream: slot (b,c) = [p5_s1(b,c); p5_s2(prev slot)] where
        # prev of (b+1,0) is (b,NCH-1) — mm4 of b's last chunk covers the ACT
        # latency of b+1's first chunk across the batch boundary.
        def gen_p2_ops(b):
            # s1 pair-ops run LAG ahead of s2 so mm2 never waits on the h1
            # relu ACT (which queues behind the wide h2 ACTs on ScalarE).
            LAG = 2
            H = NCH // 2
            ops, ready = [], []
            for k in range(H):
                ops.append(lambda b=b, k=k: p2_s1(b, k, psm))
                ready += [k, k + H]
                if k >= LAG:
                    for c in (ready.pop(0), ready.pop(0)):
                        ops.append(lambda b=b, c=c: p2_s2(b, c, psm))
            for c in ready:
                ops.append(lambda b=b, c=c: p2_s2(b, c, psm))
            return ops

        seg_ops = []
        pend = None
        ztd_next = None
        for b in range(b_loc):
            p2_ops = gen_p2_ops(b + 1) if b + 1 < b_loc else []
            target = NCH - 4
            k = 0
            fgn = [None, None]
            for c in range(NCH):
                h2 = p5_s1(b, c, ztd_cur)
                if b + 1 < b_loc and c == NCH - 1:
                    # zt MMs + the ztd add land before the last p5_s2's
                    # reduces, so ztd is ready well before (b+1, 0)'s oh MMs.
                    ztd_next = zt_compute(b + 1, fgn)
                if pend is not None:
                    p5_s2(*pend)
                    # segout of b-1 pops on late (DVE-cool) slots only
                    if seg_ops and (b + 1 >= b_loc or c >= NCH - 7):
                        seg_ops.pop(0)()
                if k < len(p2_ops) and c <= target:
                    remslots = target - c + 1
                    take = -(-(len(p2_ops) - k) // remslots)
                    for _ in range(take):
                        p2_ops[k]()
                        k += 1
                if b + 1 < b_loc:
                    if c == NCH - 3:
                        assert k == len(p2_ops), (k, len(p2_ops))
                        fgn[0] = fg_reduce(b + 1, 0)
                    elif c == NCH - 2:
                        fgn[1] = fg_reduce(b + 1, 1)
                pend = (b, c, h2)
            seg_ops = gen_seg_ops(b)
            if b + 2 < b_loc:
                load_xyz(b + 2)
            ztd_cur = ztd_next
        p5_s2(*pend)
        for op in seg_ops:
            op()

    return nc


def fold_weights(W1, b1, g1, be1, m1, v1, W2, b2, W3, b3, g2, be2, m2, v2, W4, b4):
    """Host-side constant folding. BN2's scale s2 is folded into W3c/W3g and
    its bias t2 rides along with zt (added to every zt row on device)."""
    f = np.float32
    s1 = (g1 / np.sqrt(v1 + EPS)).astype(f)
    t1 = (s1 * (b1 - m1) + be1).astype(f)
    W3g = W3[:, :256].astype(np.float64)
    W3f = W3[:, 256:].astype(np.float64)
    c3 = (b3.astype(np.float64) + (W3f + W3g) @ b2.astype(np.float64)).astype(f)
    s2 = (g2 / np.sqrt(v2 + EPS)).astype(f)
    t2 = (s2 * (c3 - m2) + be2).astype(f)
    W3c = (s2[:, None].astype(np.float64) * (W3f @ W2.astype(np.float64))).astype(f)
    W3gs = (s2[:, None] * W3g.astype(f)).astype(f)
    c4 = b4.astype(f)

    import ml_dtypes
    bf = ml_dtypes.bfloat16
    w1x = np.zeros((36, 128), f)
    w1x[0:3, :] = W1.T
    w1x[32:35, :] = W1.T
    return {
        "w1x": w1x.astype(bf),
        "w2t": np.ascontiguousarray(W2.T.astype(f)).astype(bf),
        "w3ct": np.ascontiguousarray(W3c.T).astype(bf),
        "w3gt": np.ascontiguousarray(
            W3gs.T.reshape(2, 128, 512).transpose(1, 0, 2)).astype(bf),
        "w4t": np.ascontiguousarray(
            W4.T.astype(f).reshape(4, 128, 384).transpose(1, 0, 2)).astype(bf),
        "s1t1": np.stack([s1, t1], axis=1),
        "t2r": np.broadcast_to(t2, (128, 512)).copy(),
        "c4c": np.ascontiguousarray(c4.reshape(3, 128).T),
    }


def build_ohf(Sr, starts, np2):
    import ml_dtypes
    oh = np.zeros((128, np2), np.float32)
    for r in range(K):
        oh[r, starts[r]:starts[r] + Sr[r]] = 1.0
    oh[64:128] = oh[0:64]
    return oh.astype(ml_dtypes.bfloat16)


def make_core_inputs(xyz, choice, wf, ohf, Sr, starts, np2):
    """Per-core inputs: rank-slot-sorted transposed xyz + rank orders."""
    import ml_dtypes
    b_loc = xyz.shape[0]
    xs = np.zeros((b_loc, 8, np2 // 2), ml_dtypes.bfloat16)
    orders = []
    np2_raw = int(starts[-1] + Sr[-1])
    idx = np.empty(np2_raw, np.int64)
    xt = np.zeros((4, np2), np.float32)
    for bb in range(b_loc):
        ch = choice[bb]
        cnts = np.bincount(ch, minlength=K)
        order = np.argsort(-cnts, kind="stable")
        ord_pts = np.argsort(ch, kind="stable")
        cum = np.zeros(K + 1, np.int64)
        cum[1:] = np.cumsum(cnts)
        for r in range(K):
            kcl = order[r]
            c0, c1 = cum[kcl], cum[kcl + 1]
            n = c1 - c0
            assert 1 <= n <= Sr[r], (r, n, Sr[r])
            s0 = starts[r]
            idx[s0:s0 + n] = ord_pts[c0:c1]
            idx[s0 + n:s0 + Sr[r]] = ord_pts[c1 - 1]
        xt[:3, :np2_raw] = xyz[bb][idx].T
        xs[bb, 0:4, :] = xt[:, :np2 // 2].astype(ml_dtypes.bfloat16)
        xs[bb, 4:8, :] = xt[:, np2 // 2:].astype(ml_dtypes.bfloat16)
        orders.append(order)
    m = {"xyzS": xs, "ohf": ohf}
    m.update(wf)
    return m, orders


_BUILT = {}


def get_built(tiers, np2, b_loc=B_LOC):
    key = (tiers, np2, b_loc)
    if key not in _BUILT:
        nc = bacc.Bacc("TRN2", target_bir_lowering=False, debug=False,
                       num_devices=N_CORES if b_loc == B_LOC else 1)
        build(nc, tiers, np2, b_loc)
        nc.compile()
        _BUILT[key] = nc
    return _BUILT[key]


def prepare(inputs):
    """Returns (nc, in_maps, post) where post(results) -> full output."""
    xyz = np.asarray(inputs["normalized_xyz"], np.float32)
    choice = np.asarray(inputs["choice"], np.int32)
    wf = fold_weights(**{k: np.asarray(inputs[k], np.float32) for k in
                         ["W1", "b1", "g1", "be1", "m1", "v1", "W2", "b2",
                          "W3", "b3", "g2", "be2", "m2", "v2", "W4", "b4"]})
    cnts = np.stack([np.bincount(choice[b], minlength=K) for b in range(B)])
    assert cnts.min() >= 1
    tiers, np2 = _tier_table(cnts)
    Sr = np.concatenate([[S] * n for n, S in tiers]).astype(np.int64)
    starts = np.zeros(K, np.int64)
    starts[1:] = np.cumsum(Sr)[:-1]
    nc = get_built(tiers, np2)
    ohf = build_ohf(Sr, starts, np2)
    in_maps, all_orders = [], []
    for core in range(N_CORES):
        sl = slice(core * B_LOC, (core + 1) * B_LOC)
        m, orders = make_core_inputs(xyz[sl], choice[sl], wf, ohf, Sr, starts, np2)
        in_maps.append(m)
        all_orders.append(orders)

    def post(results):
        out = np.empty((B, K, OUT), np.float32)
        for core in range(N_CORES):
            arr = np.asarray(results[core]["out"], np.float32)
            arr = arr.transpose(0, 3, 1, 2).reshape(B_LOC, K, OUT)
            for bb in range(B_LOC):
                out[core * B_LOC + bb, all_orders[core][bb]] = arr[bb]
        return out

    return nc, in_maps, post


def kernel(**inputs):
    from concourse.bass_utils import run_bass_kernel_spmd
    nc, in_maps, post = prepare(inputs)
    res = run_bass_kernel_spmd(nc, in_maps, core_ids=list(range(N_CORES)))
    return post(res.results)
